# revision 3
# baseline (speedup 1.0000x reference)
"""Bass/Trainium2 kernel for nn_BayesianResNet_71408126263673.

Grouped per-sample conv: for each of 32 samples i,
  out[i] = conv2d(x[i] [128,32,32], W[i] [128oc,128c,3,3], pad=1, stride=1) + bias[i]

Sharding: b_i (32 samples) split across 8 NeuronCores, 4 samples per core.
Pure data parallel, no collectives.

Per-core kernel: each sample's conv is computed as 9 accumulating matmuls
(one per 3x3 tap) into PSUM:
  out[oc, pix] = sum_{kh,kw} W[:, :, kh, kw].T @ xpad[:, shifted pix]
with K=c=128 (partition/contraction), M=oc=128, N<=512 pixels per PSUM bank.
The input image is zero-padded to 34x34 on the HOST so DMA loads are fully
contiguous. Weights are pre-transposed on the host to [c, kh*kw, oc] so each
tap is a ready-to-use lhsT (stationary operand) tile.

Timeline engineering (from perfetto traces):
- Sample 0's load is split in three so block-0 compute is gated only on
  [9 weight taps | image rows 0..17]: weights ride the fast SP HWDGE queue,
  the first image rows ride the GpSimd SWDGE queue in parallel, the rest
  follows. First real matmul ~8.9us instead of ~11.4us.
- A short PE warmup (dep-free matmuls on garbage) keeps the HAM clock-gate
  fed from engine start; real matmuls begin cold (~1.2GHz) as soon as data
  lands and go 2.4GHz once the ~3.4us activity window fills.
- Outputs are written fp16 (host upcasts): halves store bytes; the last
  sample is split 16/8/8 rows so the final ACT+store tail is small.
"""

import numpy as np

import concourse.bacc as bacc
import concourse.tile as tile
from concourse import mybir
from concourse.bass_utils import run_bass_kernel_spmd

N_CORES = 8
B_I, B_J, C, H, W = 32, 1, 128, 32, 32
OC, KH, KW = 128, 3, 3
S = B_I // N_CORES            # samples per core
HP, WP = H + 2, W + 2         # padded image
NTAP = KH * KW                # 9
WLEN = NTAP * OC              # 1152 weight columns per sample
XLEN = HP * WP                # 1156 image columns per sample

MM_DT = mybir.dt.float16
MM_NP = np.float16
OUT_DT = mybir.dt.float16
OUT_NP = np.float16
X_DT = W_DT = MM_DT  # test.py prints these

# Row-block split per sample: 16+16, except the last sample 16+8+8 so the
# final ACT+store (the serial tail after the last matmul) is half-sized.
BLOCKS = [(0, 16), (16, 16)]
BLOCKS_LAST = [(0, 16), (16, 8), (24, 8)]

N_WARMUP = 18  # ~1.9us of N=128 cold matmuls; bridges engine start -> data

# test.py hooks: set TRACE=True before calling kernel() to profile; the
# BassKernelResults of the last run lands in LAST_RESULTS.
TRACE = False
TRACE_KW = {}
LAST_RESULTS = None

_NC_CACHE = None

# Split of sample 0's load: block 0 needs all taps + image rows 0..17.
R0_ROWS = 18                          # rows 0..17 (kh reach of block 0)
A_END = WLEN                          # taps chunk      [0, 1152)   (SP queue)
R0_END = WLEN + R0_ROWS * WP          # rows 0..17      [1152, 1764) (SWDGE)
# remainder rows 18..33               [1764, 2308)    (SP queue)


def _build_nc():
    f32 = mybir.dt.float32
    nc = bacc.Bacc()
    xw_d = nc.declare_dram_parameter(
        "xw", [S, C, WLEN + XLEN], MM_DT, isOutput=False
    )
    b_d = nc.declare_dram_parameter("b", [OC, S], f32, isOutput=False)
    o_d = nc.declare_dram_parameter("o", [S, OC, H, W], OUT_DT, isOutput=True)

    with tile.TileContext(nc, pool_alloc_mode="queue") as tc:
        with (
            tc.tile_pool(name="ins", bufs=1) as ins_pool,
            tc.tile_pool(name="outs", bufs=1) as outs_pool,
            tc.tile_pool(name="psum", bufs=8, space="PSUM") as psum_pool,
        ):
            # PE warmup: dependency-free matmuls on garbage data keep the PE
            # busy from engine start so the HAM clock-gate reaches 2.4 GHz
            # ~3.4us later. Their PSUM tile is never read.
            wu_x = ins_pool.tile([C, OC], MM_DT, tag="warmup", name="warmup")
            nc.vector.memset(wu_x[:], 0.0)
            wu_ps = psum_pool.tile([C, OC], f32, name="wu_ps", tag="ps")
            for _ in range(N_WARMUP):
                nc.tensor.matmul(wu_ps[:], wu_x[:], wu_x[:], start=True, stop=True)

            xw_ts = [
                ins_pool.tile(
                    [C, WLEN + XLEN], MM_DT, tag=f"xw{s}", name=f"xw{s}"
                )
                for s in range(S)
            ]
            wts = [t[:, :WLEN] for t in xw_ts]
            xvs = [
                t[:, WLEN:].rearrange("p (h w) -> p h w", w=WP) for t in xw_ts
            ]
            bias_t = ins_pool.tile([OC, S], f32, tag="bias")

            # Sample 0 split across SP HWDGE (fast ramp) + GpSimd SWDGE so
            # the two chunks gating block 0 stream in parallel from t~0.
            nc.sync.dma_start(xw_ts[0][:, :A_END], xw_d[0][:, :A_END])
            nc.gpsimd.dma_start(
                xw_ts[0][:, A_END:R0_END], xw_d[0][:, A_END:R0_END]
            )
            nc.scalar.dma_start(bias_t[:], b_d[:])  # tiny; warms the ACT queue
            nc.sync.dma_start(xw_ts[0][:, R0_END:], xw_d[0][:, R0_END:])
            # Samples 1-3: whole-sample loads. SP's queue drains ~200+ GB/s,
            # deadlines are ~4us apart per sample, so SP carries 1 and 2;
            # the slow-ramping ACT queue gets sample 3 (needed last).
            nc.sync.dma_start(xw_ts[1][:], xw_d[1])
            nc.sync.dma_start(xw_ts[2][:], xw_d[2])
            nc.scalar.dma_start(xw_ts[3][:], xw_d[3])

            def conv_block(s, row0, nrows, ps_name):
                """One accumulation group: output rows [row0, row0+nrows)."""
                ps = psum_pool.tile([OC, nrows, W], f32, name=ps_name, tag="ps")
                for t in range(NTAP):
                    kh, kw = divmod(t, KW)
                    rhs = xvs[s][:, row0 + kh : row0 + kh + nrows, kw : kw + W]
                    lhsT = wts[s][:, t * OC : (t + 1) * OC]
                    nc.tensor.matmul(
                        ps[:], lhsT, rhs, start=(t == 0), stop=(t == NTAP - 1)
                    )
                return ps

            for s in range(S):
                out_t = outs_pool.tile(
                    [OC, H, W], OUT_DT, tag=f"out{s}", name=f"out{s}"
                )
                blocks = BLOCKS_LAST if s == S - 1 else BLOCKS
                for bi, (row0, nrows) in enumerate(blocks):
                    ps = conv_block(s, row0, nrows, f"ps{s}_{bi}")
                    nc.scalar.activation(
                        out_t[:, row0 : row0 + nrows, :],
                        ps[:],
                        mybir.ActivationFunctionType.Identity,
                        bias=bias_t[:, s : s + 1],
                    )
                    # Store each block as soon as its ACT lands. Inputs own
                    # the SP ring early on, so early stores ride ACT's ring;
                    # by the tail both rings are free and the last two
                    # (small) blocks go out in parallel on both.
                    if s == S - 1 and bi == len(blocks) - 2:
                        eng = nc.sync
                    else:
                        eng = nc.scalar
                    eng.dma_start(
                        o_d[s][:, row0 : row0 + nrows, :],
                        out_t[:, row0 : row0 + nrows, :],
                    )
    nc.compile()
    return nc


def _get_nc():
    global _NC_CACHE
    if _NC_CACHE is None:
        _NC_CACHE = _build_nc()
    return _NC_CACHE


def kernel(x: np.ndarray, weight: np.ndarray, bias: np.ndarray) -> np.ndarray:
    global LAST_RESULTS
    assert x.shape == (B_I, B_J, C, H, W)
    assert weight.shape == (B_I, OC, C, KH, KW)
    assert bias.shape == (B_I, B_J, OC)

    x = np.asarray(x, dtype=np.float32)
    weight = np.asarray(weight, dtype=np.float32)
    bias = np.asarray(bias, dtype=np.float32)

    # Host-side layout prep (part of sharding): zero-pad images, transpose
    # weights so each 3x3 tap is a contiguous [c, oc] stationary tile.
    xw = np.zeros((B_I, C, WLEN + XLEN), dtype=MM_NP)
    wt = np.ascontiguousarray(weight.transpose(0, 2, 3, 4, 1))  # [b_i, c, kh, kw, oc]
    xw[:, :, :WLEN] = wt.reshape(B_I, C, WLEN).astype(MM_NP)
    xpad = xw[:, :, WLEN:].reshape(B_I, C, HP, WP)
    xpad[:, :, 1 : 1 + H, 1 : 1 + W] = x[:, 0].astype(MM_NP)
    bt = bias[:, 0, :]  # [b_i, oc]

    in_maps = []
    for core in range(N_CORES):
        sl = slice(core * S, (core + 1) * S)
        in_maps.append(
            {
                "xw": np.ascontiguousarray(xw[sl]),
                "b": np.ascontiguousarray(bt[sl].T),  # [OC, S]
            }
        )

    nc = _get_nc()
    try:
        res = run_bass_kernel_spmd(
            nc, in_maps, core_ids=list(range(N_CORES)), trace=TRACE, **TRACE_KW
        )
    except Exception:
        # Transient NRT/device errors (e.g. NRT_EXEC_UNIT_UNRECOVERABLE after
        # heavy reuse) usually clear on retry; the work is idempotent.
        import time

        time.sleep(10)
        res = run_bass_kernel_spmd(
            nc, in_maps, core_ids=list(range(N_CORES)), trace=TRACE, **TRACE_KW
        )
    LAST_RESULTS = res

    out = np.concatenate(
        [res.results[c]["o"].astype(np.float32) for c in range(N_CORES)], axis=0
    )
    return out.reshape(B_I, B_J, OC, H, W)


# revision 4
# speedup vs baseline: 1.0005x; 1.0005x over previous
"""Bass/Trainium2 kernel for nn_BayesianResNet_71408126263673.

Grouped per-sample conv: for each of 32 samples i,
  out[i] = conv2d(x[i] [128,32,32], W[i] [128oc,128c,3,3], pad=1, stride=1) + bias[i]

Sharding: b_i (32 samples) split across 8 NeuronCores, 4 samples per core.
Pure data parallel, no collectives.

Per-core kernel: each sample's conv is computed as 9 accumulating matmuls
(one per 3x3 tap) into PSUM:
  out[oc, pix] = sum_{kh,kw} W[:, :, kh, kw].T @ xpad[:, shifted pix]
with K=c=128 (partition/contraction), M=oc=128, N<=512 pixels per PSUM bank.
The input image is zero-padded to 34x34 on the HOST so DMA loads are fully
contiguous. Weights are pre-transposed on the host to [c, kh*kw, oc] so each
tap is a ready-to-use lhsT (stationary operand) tile.

Timeline engineering (from perfetto traces):
- Per-sample SBUF layout is [taps 0-2 | img rows 0-17 | taps 3-8 |
  img rows 16-33] (rows 16/17 duplicated) so sample 0 streams in as three
  sequential chunks on the fast SP HWDGE queue and Tile's address-range
  dependency tracking releases each matmul as soon as ITS tap/rows land:
  first real matmul ~9.6us instead of ~11.4us.
- PE warmup (dep-free matmuls on garbage) runs from engine start until the
  first data lands with NO idle gap: the HAM clock-gate needs ~3.4us of
  continuous PE activity to lift the 1.2->2.4GHz throttle, and any idle gap
  restarts that window (measured: a 1.7us gap costs ~1.5us of cold matmuls).
- Outputs are written fp16 (host upcasts): halves store bytes; the last
  sample is split 16/8/8 rows so the final ACT+store tail is small.
"""

import numpy as np

import concourse.bacc as bacc
import concourse.tile as tile
from concourse import mybir
from concourse.bass_utils import run_bass_kernel_spmd

N_CORES = 8
B_I, B_J, C, H, W = 32, 1, 128, 32, 32
OC, KH, KW = 128, 3, 3
S = B_I // N_CORES            # samples per core
HP, WP = H + 2, W + 2         # padded image
NTAP = KH * KW                # 9

MM_DT = mybir.dt.float16
MM_NP = np.float16
OUT_DT = mybir.dt.float16
X_DT = W_DT = MM_DT  # test.py prints these

# Per-sample column layout (partition dim = C):
#   [taps 0-2 | rows 0-17 | taps 3-8 | rows 16-33]
NT_A = 3                      # taps in the first segment
ROWS_A = 18                   # rows 0..17  (covers block-0 reach)
ROWS_B = 18                   # rows 16..33 (covers block-1 reach)
SEG0 = 0
SEG1 = SEG0 + NT_A * OC       # 384:  rows 0-17 start
SEG2 = SEG1 + ROWS_A * WP     # 996:  taps 3-8 start
SEG3 = SEG2 + (NTAP - NT_A) * OC  # 1764: rows 16-33 start
NCOL = SEG3 + ROWS_B * WP     # 2376 columns total

# Sample-0 DMA chunk boundaries (sequential on the SP queue).
CH1 = SEG2                    # taps 0-2 + rows 0-17
CH2 = SEG3                    # taps 3-8

# Row-block split per sample: 16+16, except the last sample 16+8+8 so the
# final ACT+store (the serial tail after the last matmul) is half-sized.
BLOCKS = [(0, 16), (16, 16)]
BLOCKS_LAST = [(0, 16), (16, 8), (24, 8)]

N_WARMUP = 26  # ~2.8us of N=128 cold matmuls; bridges engine start -> data

# test.py hooks
TRACE = False
TRACE_KW = {}
LAST_RESULTS = None

_NC_CACHE = None


def _build_nc():
    f32 = mybir.dt.float32
    nc = bacc.Bacc()
    xw_d = nc.declare_dram_parameter("xw", [S, C, NCOL], MM_DT, isOutput=False)
    b_d = nc.declare_dram_parameter("b", [OC, S], f32, isOutput=False)
    o_d = nc.declare_dram_parameter("o", [S, OC, H, W], OUT_DT, isOutput=True)

    with tile.TileContext(nc, pool_alloc_mode="queue") as tc:
        with (
            tc.tile_pool(name="ins", bufs=1) as ins_pool,
            tc.tile_pool(name="outs", bufs=1) as outs_pool,
            tc.tile_pool(name="psum", bufs=8, space="PSUM") as psum_pool,
        ):
            wu_x = ins_pool.tile([C, OC], MM_DT, tag="warmup", name="warmup")
            nc.gpsimd.memset(wu_x[:], 0.0)
            wu_ps = psum_pool.tile([C, OC], f32, name="wu_ps", tag="ps")
            for _ in range(N_WARMUP):
                nc.tensor.matmul(wu_ps[:], wu_x[:], wu_x[:], start=True, stop=True)

            xw_ts = [
                ins_pool.tile([C, NCOL], MM_DT, tag=f"xw{s}", name=f"xw{s}")
                for s in range(S)
            ]
            bias_t = ins_pool.tile([OC, S], f32, tag="bias")

            def tap_view(s, t):
                if t < NT_A:
                    return xw_ts[s][:, t * OC : (t + 1) * OC]
                return xw_ts[s][:, SEG2 + (t - NT_A) * OC : SEG2 + (t - NT_A + 1) * OC]

            # image views: rows 0-17 and rows 16-33 (as local rows 0-17)
            xva = [
                t[:, SEG1:SEG2].rearrange("p (h w) -> p h w", w=WP) for t in xw_ts
            ]
            xvb = [
                t[:, SEG3:].rearrange("p (h w) -> p h w", w=WP) for t in xw_ts
            ]

            # Sample 0: three sequential chunks on the SP queue; matmuls are
            # released per-chunk by Tile's range tracking.
            nc.sync.dma_start(xw_ts[0][:, :CH1], xw_d[0][:, :CH1])
            nc.scalar.dma_start(bias_t[:], b_d[:])  # tiny; warms the ACT queue
            nc.sync.dma_start(xw_ts[0][:, CH1:CH2], xw_d[0][:, CH1:CH2])
            nc.sync.dma_start(xw_ts[0][:, CH2:], xw_d[0][:, CH2:])
            # Samples 1-3: whole-sample loads. ACT's queue ramps slowly
            # (~3us to first byte) but sample 1 isn't needed until ~13.5us.
            nc.scalar.dma_start(xw_ts[1][:], xw_d[1])
            nc.sync.dma_start(xw_ts[2][:], xw_d[2])
            nc.scalar.dma_start(xw_ts[3][:], xw_d[3])

            def conv_block(s, row0, nrows, ps_name):
                """One accumulation group: output rows [row0, row0+nrows)."""
                ps = psum_pool.tile([OC, nrows, W], f32, name=ps_name, tag="ps")
                xv, base = (xva[s], 0) if row0 + nrows + 2 <= ROWS_A else (xvb[s], 16)
                for t in range(NTAP):
                    kh, kw = divmod(t, KW)
                    r0 = row0 - base + kh
                    rhs = xv[:, r0 : r0 + nrows, kw : kw + W]
                    nc.tensor.matmul(
                        ps[:], tap_view(s, t), rhs,
                        start=(t == 0), stop=(t == NTAP - 1),
                    )
                return ps

            for s in range(S):
                out_t = outs_pool.tile(
                    [OC, H, W], OUT_DT, tag=f"out{s}", name=f"out{s}"
                )
                blocks = BLOCKS_LAST if s == S - 1 else BLOCKS
                for bi, (row0, nrows) in enumerate(blocks):
                    ps = conv_block(s, row0, nrows, f"ps{s}_{bi}")
                    nc.scalar.activation(
                        out_t[:, row0 : row0 + nrows, :],
                        ps[:],
                        mybir.ActivationFunctionType.Identity,
                        bias=bias_t[:, s : s + 1],
                    )
                    # Store each block as soon as its ACT lands; the last two
                    # (small) blocks go out in parallel on both queues.
                    eng = nc.sync if (s == S - 1 and bi == len(blocks) - 2) else nc.scalar
                    eng.dma_start(
                        o_d[s][:, row0 : row0 + nrows, :],
                        out_t[:, row0 : row0 + nrows, :],
                    )
    nc.compile()
    return nc


def _get_nc():
    global _NC_CACHE
    if _NC_CACHE is None:
        _NC_CACHE = _build_nc()
    return _NC_CACHE


def kernel(x: np.ndarray, weight: np.ndarray, bias: np.ndarray) -> np.ndarray:
    global LAST_RESULTS
    assert x.shape == (B_I, B_J, C, H, W)
    assert weight.shape == (B_I, OC, C, KH, KW)
    assert bias.shape == (B_I, B_J, OC)

    x = np.asarray(x, dtype=np.float32)
    weight = np.asarray(weight, dtype=np.float32)
    bias = np.asarray(bias, dtype=np.float32)

    # Host-side layout prep (part of sharding): zero-pad images, transpose
    # weights so each 3x3 tap is a contiguous [c, oc] stationary tile.
    # Layout per sample: [taps 0-2 | rows 0-17 | taps 3-8 | rows 16-33].
    wt = weight.transpose(0, 2, 3, 4, 1).reshape(B_I, C, NTAP * OC).astype(MM_NP)
    xpad = np.zeros((B_I, C, HP, WP), dtype=MM_NP)
    xpad[:, :, 1 : 1 + H, 1 : 1 + W] = x[:, 0].astype(MM_NP)

    xw = np.empty((B_I, C, NCOL), dtype=MM_NP)
    xw[:, :, SEG0:SEG1] = wt[:, :, : NT_A * OC]
    xw[:, :, SEG1:SEG2] = xpad[:, :, :ROWS_A].reshape(B_I, C, ROWS_A * WP)
    xw[:, :, SEG2:SEG3] = wt[:, :, NT_A * OC :]
    xw[:, :, SEG3:] = xpad[:, :, HP - ROWS_B :].reshape(B_I, C, ROWS_B * WP)
    bt = bias[:, 0, :]  # [b_i, oc]

    in_maps = []
    for core in range(N_CORES):
        sl = slice(core * S, (core + 1) * S)
        in_maps.append(
            {
                "xw": np.ascontiguousarray(xw[sl]),
                "b": np.ascontiguousarray(bt[sl].T),  # [OC, S]
            }
        )

    nc = _get_nc()
    try:
        res = run_bass_kernel_spmd(
            nc, in_maps, core_ids=list(range(N_CORES)), trace=TRACE, **TRACE_KW
        )
    except Exception:
        # Transient NRT/device errors usually clear on retry; idempotent.
        import time

        time.sleep(10)
        res = run_bass_kernel_spmd(
            nc, in_maps, core_ids=list(range(N_CORES)), trace=TRACE, **TRACE_KW
        )
    LAST_RESULTS = res

    out = np.concatenate(
        [res.results[c]["o"].astype(np.float32) for c in range(N_CORES)], axis=0
    )
    return out.reshape(B_I, B_J, OC, H, W)
